# revision 10
# baseline (speedup 1.0000x reference)
"""Causal self-attention (B=4, T=2048, C=1024, H=16) on 8 trn2 NeuronCores.

Sharding: tensor-parallel over heads. Core c owns heads {2c, 2c+1}:
  - computes Q,K,V projections for its 2 heads (full batch/sequence),
  - causal attention for its heads,
  - a partial output projection (row-slice of W_proj),
and the host sums the 8 partial projections (+ b_proj).

This version is built around keeping the PE (tensor engine) stream stall-free
so it ramps to and stays at full clock (back-to-back 512-row fp32r matmuls
measure ~230ns on this part):

  - All matmuls are fp32r (1 cycle/row at N>=256; bf16 is no faster on this
    hardware, measured) except the output staging, which is written bf16 to
    halve the output DMA (accuracy measured at ~1.7e-3 l2 vs 2e-2 budget).
  - Attention is software-pipelined: S^T of k-tile i+1 is issued before AV of
    k-tile i, so the exp (Activation engine) of tile i overlaps the S matmuls
    of tile i+1 on the PE.
  - Both heads are packed in one [128, 2, 512] S^T/P tile -> one exp per
    k-tile covers both heads.
  - Causal masking of diagonal tiles is done by zeroing the upper triangle of
    P AFTER exp with gpsimd affine_select (gpsimd is otherwise idle), keeping
    mask work off the DVE and scalar engines.
  - The softmax denominator l comes from a shared ones-column in the
    token-major V tile (layout [ones, V_h0, V_h1, ones], 130 cols): head0's
    AV uses cols 0:65 -> l lands on psum partition 63, O on 64:127; head1's
    uses cols 65:130 -> O on 0:63, l on 64. No cross-partition moves needed.
  - 1/l via the fast approximate-reciprocal DVE op on the single l row, then
    a K=1 ones-matmul broadcast across partitions.
  - QKV projection for batch b+1 is emitted between the attention windows of
    batch b: it fills the PE while the window-end normalize chain (DVE) runs.
  - The per-window output projection is split into 4 deferred pieces that are
    drip-fed into the next window's k-loop (1 piece per k-tile iteration), so
    the PSUM->SBUF staging copies never gate the PE.
  - PSUM budget (8 banks): stt ring 2x[128,2,512] (4) + av_h0/av_h1 (2) +
    po ring 2x[128,512] (2). The normalize broadcast and the V transposes
    borrow stt-ring slots.
"""

import numpy as np

import concourse.bacc as bacc
import concourse.bass as bass
import concourse.tile as tile
from concourse import mybir
from concourse.bass_utils import run_bass_kernel_spmd
from concourse.masks import make_identity

F32 = mybir.dt.float32
F32R = mybir.dt.float32r
BF16 = mybir.dt.bfloat16
AF = mybir.ActivationFunctionType
ALU = mybir.AluOpType

N_CORES = 8
D_MODEL = 1024
HEAD_DIM = 64
H_LOC = 2                  # heads per core
D_LOC = H_LOC * HEAD_DIM   # 128
SCALE = 1.0 / np.sqrt(HEAD_DIM)


def build_program(B=4, T=2048):
    TOK = B * T
    CT = D_MODEL // 128    # contraction tiles for the QKV matmul
    NTT = T // 512         # 512-token tiles per batch
    NW = T // 512          # q-windows per batch (512 wide)
    NKB = T // 128         # k-tiles per batch
    assert T % 512 == 0

    nc = bacc.Bacc(
        "TRN2", target_bir_lowering=False, debug=False, num_devices=N_CORES
    )
    xT = nc.dram_tensor("xT", [D_MODEL, TOK], F32R, kind="ExternalInput").ap()
    wq = nc.dram_tensor("wq", [D_MODEL, D_LOC], F32R, kind="ExternalInput").ap()
    wk = nc.dram_tensor("wk", [D_MODEL, D_LOC], F32R, kind="ExternalInput").ap()
    wv = nc.dram_tensor("wv", [D_MODEL, D_LOC], F32R, kind="ExternalInput").ap()
    bq = nc.dram_tensor("bq", [D_LOC, 1], F32, kind="ExternalInput").ap()
    bk = nc.dram_tensor("bk", [D_LOC, 1], F32, kind="ExternalInput").ap()
    bv = nc.dram_tensor("bv", [D_LOC, 1], F32, kind="ExternalInput").ap()
    # wp rows are pre-swapped on host: [head1 dims, head0 dims]
    wp = nc.dram_tensor("wp", [D_LOC, D_MODEL], F32R, kind="ExternalInput").ap()
    outp = nc.dram_tensor("outp", [TOK, D_MODEL], BF16, kind="ExternalOutput").ap()

    with tile.TileContext(nc) as tc:
        with (
            tc.tile_pool(name="const", bufs=1) as const,
            tc.tile_pool(name="res", bufs=1) as res,
            tc.tile_pool(name="xp", bufs=3) as xp,
            tc.tile_pool(name="vtp", bufs=2) as vtp,
            tc.tile_pool(name="ptp", bufs=3) as ptp,
            tc.tile_pool(name="lnp", bufs=2) as lnp,
            tc.tile_pool(name="obp", bufs=4) as obp,
            tc.tile_pool(name="pst", bufs=1, space="PSUM") as pst,
            tc.tile_pool(name="pav", bufs=1, space="PSUM") as pav,
            tc.tile_pool(name="ppo", bufs=1, space="PSUM") as ppo,
        ):
            # --- constants -------------------------------------------------
            wq_sb = const.tile([128, CT, D_LOC], F32R, tag="wq")
            wk_sb = const.tile([128, CT, D_LOC], F32R, tag="wk")
            wv_sb = const.tile([128, CT, D_LOC], F32R, tag="wv")
            for w_sb, w_dram in ((wq_sb, wq), (wk_sb, wk), (wv_sb, wv)):
                nc.sync.dma_start(
                    out=w_sb, in_=w_dram.rearrange("(ct p) d -> p ct d", p=128)
                )
            wp_sb = const.tile([128, D_MODEL], F32R, tag="wp")
            nc.sync.dma_start(out=wp_sb, in_=wp)
            bq_sb = const.tile([128, 1], F32, tag="bq")
            bk_sb = const.tile([128, 1], F32, tag="bk")
            bv_sb = const.tile([128, 1], F32, tag="bv")
            for b_sb, b_dram in ((bq_sb, bq), (bk_sb, bk), (bv_sb, bv)):
                nc.sync.dma_start(out=b_sb, in_=b_dram)

            ident_f32 = const.tile([128, 128], F32, tag="ident_f32")
            make_identity(nc, ident_f32)
            ident = const.tile([128, 128], F32R, tag="ident")
            nc.vector.tensor_copy(ident, ident_f32)
            ones_f32 = const.tile([128, 128], F32, tag="ones_f32")
            nc.vector.memset(ones_f32, 1.0)
            ones_r = const.tile([128, 128], F32R, tag="ones_r")
            nc.vector.tensor_copy(ones_r, ones_f32)
            # 0/1 causal mask for diagonal S^T blocks: keep where q - k >= 0
            tri_f32 = const.tile([128, 128], F32, tag="tri_f32")
            nc.vector.memset(tri_f32, 1.0)
            nc.gpsimd.affine_select(
                out=tri_f32,
                in_=tri_f32,
                compare_op=ALU.is_ge,
                fill=0.0,
                base=0,
                pattern=[[1, 128]],
                channel_multiplier=-1,
            )
            trimask = const.tile([128, 128], F32R, tag="trimask")
            nc.vector.tensor_copy(trimask, tri_f32)

            # --- per-batch residents (double-buffered) ---------------------
            qt = [res.tile([128, T], F32R, tag=f"qt{i}", name=f"qt{i}") for i in range(2)]
            kt = [res.tile([128, T], F32R, tag=f"kt{i}", name=f"kt{i}") for i in range(2)]
            # token-major V: [tok, kblk, 130] = [V_h0 (64), ones, V_h1 (64), ones]
            vtm = [res.tile([128, NKB, 130], F32R, tag=f"vtm{i}", name=f"vtm{i}") for i in range(2)]
            ot = [res.tile([128, T], F32R, tag=f"ot{i}", name=f"ot{i}") for i in range(2)]
            for i in range(2):
                nc.vector.tensor_copy(vtm[i][:, :, 64], ones_f32[:, 0:NKB])
                nc.vector.tensor_copy(vtm[i][:, :, 129], ones_f32[:, 0:NKB])

            x_tiles = {}

            def emit_x_load(b, tt):
                g = b * NTT + tt
                xt = xp.tile([128, CT, 512], F32R, tag="x", name=f"x_{g}")
                t0 = b * T + tt * 512
                for ct in range(CT):
                    nc.sync.dma_start(
                        out=xt[:, ct, :],
                        in_=xT[ct * 128:(ct + 1) * 128, t0:t0 + 512],
                    )
                x_tiles[g] = xt

            def st_tile(name):
                return pst.tile([128, 512], F32, tag="st", bufs=4, name=name)

            def po_tile(name):
                return pst.tile([128, 2, 512], F32, tag="po", bufs=1, name=name)

            def emit_qkv_qk(b, tt):
                """Q/K matmuls + bias adds for one 512-token tile."""
                bb = b % 2
                xt = x_tiles[b * NTT + tt]
                t0 = tt * 512
                ta = po_tile(f"qkvA_{b}_{tt}")
                for ct in range(CT):
                    nc.tensor.matmul(
                        ta[:, 0, :], wq_sb[:, ct, :], xt[:, ct, :],
                        start=(ct == 0), stop=(ct == CT - 1),
                    )
                nc.vector.tensor_scalar_add(qt[bb][:, t0:t0 + 512], ta[:, 0, :], bq_sb)
                for ct in range(CT):
                    nc.tensor.matmul(
                        ta[:, 1, :], wk_sb[:, ct, :], xt[:, ct, :],
                        start=(ct == 0), stop=(ct == CT - 1),
                    )
                nc.vector.tensor_scalar_add(kt[bb][:, t0:t0 + 512], ta[:, 1, :], bk_sb)

            def emit_qkv_v(b, tt):
                """V matmuls, bias, transpose into token-major vtm."""
                bb = b % 2
                xt = x_tiles.pop(b * NTT + tt)
                tb = po_tile(f"qkvB_{b}_{tt}")
                for ct in range(CT):
                    nc.tensor.matmul(
                        tb[:, 0, :], wv_sb[:, ct, :], xt[:, ct, :],
                        start=(ct == 0), stop=(ct == CT - 1),
                    )
                vt = vtp.tile([128, 512], F32R, tag="vt", name=f"vt_{b}_{tt}")
                nc.vector.tensor_scalar_add(vt, tb[:, 0, :], bv_sb)
                st = st_tile(f"tp_{b}_{tt}")
                for j in range(4):
                    nc.tensor.transpose(
                        st[:, j * 128:(j + 1) * 128].bitcast(F32R),
                        vt[:, j * 128:(j + 1) * 128],
                        ident,
                    )
                tpv = st.bitcast(F32R).rearrange("p (j q) -> p j q", j=4)
                nc.vector.tensor_copy(
                    vtm[bb][:, tt * 4:(tt + 1) * 4, 0:64], tpv[:, :, 0:64]
                )
                nc.vector.tensor_copy(
                    vtm[bb][:, tt * 4:(tt + 1) * 4, 65:129], tpv[:, :, 64:128]
                )

            deferred = []

            def push_proj(b, w):
                bb = b % 2
                for ti in range(4):
                    def piece(b=b, w=w, ti=ti, bb=bb):
                        ob = obp.tile([128, 1024], BF16, tag="ob",
                                      name=f"ob_{b}_{w}_{ti}")
                        t0 = w * 512 + ti * 128
                        t = po_tile(f"po_{b}_{w}_{ti}")
                        for co in range(2):
                            nc.tensor.matmul(
                                t[:, co, :],
                                ot[bb][:, t0:t0 + 128],
                                wp_sb[:, co * 512:(co + 1) * 512],
                                start=True, stop=True,
                            )
                        # stage to SBUF bf16, split across DVE and ACT
                        nc.vector.tensor_copy(ob[:, 0:512], t[:, 0, :])
                        nc.scalar.copy(ob[:, 512:1024], t[:, 1, :])
                        nc.gpsimd.dma_start(
                            out=outp[b * T + t0:b * T + t0 + 128, :], in_=ob
                        )
                    deferred.append(piece)

            def emit_window(b, w):
                """S/exp/AV loop for q-window [w*512, (w+1)*512) of batch b.
                Per-head S tiles and exps keep the exp latency off the PE
                critical path with only 1-bank psum slots."""
                bb = b % 2
                nk = 4 * (w + 1)
                q0 = w * 512
                av = pst.tile([128, 2, 512], F32, tag="av", bufs=1,
                              name=f"av_{b}_{w}")

                def emit_av(ki, off, pt):
                    stf, spf = (ki == 0), (ki == nk - 1)
                    nc.tensor.matmul(
                        av[0:65, 0, off:512],
                        vtm[bb][:, ki, 0:65],
                        pt[:, 0, off:512],
                        start=stf, stop=spf,
                    )
                    nc.tensor.matmul(
                        av[0:65, 1, off:512],
                        vtm[bb][:, ki, 65:130],
                        pt[:, 1, off:512],
                        start=stf, stop=spf,
                    )

                prev = None
                for ki in range(nk):
                    off = max(0, ki * 128 - q0)
                    diag = ki * 128 >= q0
                    pt = ptp.tile([128, 2, 512], F32R, tag="pt",
                                  name=f"pt_{b}_{w}_{ki}")
                    for h in range(2):
                        sth = st_tile(f"st_{b}_{w}_{ki}_{h}")
                        nc.tensor.matmul(
                            sth[:, off:512],
                            kt[bb][64 * h:64 * h + 64, ki * 128:(ki + 1) * 128],
                            qt[bb][64 * h:64 * h + 64, q0 + off:q0 + 512],
                            start=True, stop=True,
                        )
                        nc.scalar.activation(
                            pt[:, h, off:512], sth[:, off:512], AF.Exp,
                            scale=SCALE,
                        )
                        if diag:  # diagonal block: zero where q < k
                            nc.gpsimd.tensor_mul(
                                pt[:, h, off:off + 128],
                                pt[:, h, off:off + 128],
                                trimask,
                            )
                    if prev is not None:
                        emit_av(*prev)
                    prev = (ki, off, pt)
                    if deferred:
                        deferred.pop(0)()
                emit_av(*prev)

                lrow = lnp.tile([128, 2, 512], F32R, tag="lr", name=f"lr_{b}_{w}")
                nc.vector.tensor_copy(lrow[64:65, 0, :], av[64:65, 0, :])
                nc.vector.tensor_copy(lrow[64:65, 1, :], av[64:65, 1, :])
                return av, lrow

            def emit_bc(b, w, lrow):
                # broadcast l to partitions 0:64 (slot n1: head1, n2: head0)
                st_n1 = st_tile(f"stn1_{b}_{w}")
                st_n2 = st_tile(f"stn2_{b}_{w}")
                nc.tensor.matmul(
                    st_n1[0:64, :], ones_r[64:65, 0:64], lrow[64:65, 1, :],
                    start=True, stop=True,
                )
                nc.tensor.matmul(
                    st_n2[0:64, :], ones_r[64:65, 0:64], lrow[64:65, 0, :],
                    start=True, stop=True,
                )
                return st_n1, st_n2

            def emit_norm_finish(b, w, av, st_n1, st_n2):
                bb = b % 2
                q0 = w * 512
                linv_sb = lnp.tile([128, 2, 512], F32, tag="ls", name=f"ls_{b}_{w}")
                nc.vector.reciprocal_approx_fast(
                    out=linv_sb[0:64, 0, :], in_=st_n1[0:64, :]
                )
                nc.vector.reciprocal_approx_fast(
                    out=linv_sb[0:64, 1, :], in_=st_n2[0:64, :]
                )
                nc.vector.tensor_mul(
                    ot[bb][0:64, q0:q0 + 512], av[0:64, 1, :], linv_sb[0:64, 0, :]
                )
                stg = lnp.tile([64, 512], F32R, tag="stg", name=f"stg_{b}_{w}")
                nc.vector.tensor_mul(
                    stg, av[0:64, 0, :], linv_sb[0:64, 1, :]
                )
                # cross-partition move: head0 O -> ot partitions 64:128,
                # split into 4 DMAs so the first proj piece isn't gated long
                for j in range(4):
                    nc.sync.dma_start(
                        out=ot[bb][64:128, q0 + j * 128:q0 + (j + 1) * 128],
                        in_=stg[:, j * 128:(j + 1) * 128],
                    )

            # ===================== schedule ================================
            # QKV unit g = 4*b + tt is consumed just before it is first
            # needed: unit 0 in the prologue, unit g at the end of attention
            # window g-1 (window (b, w) only reads tts 0..w of batch b).
            order = [(b, tt) for b in range(B) for tt in range(NTT)]
            for i in range(min(3, len(order))):
                emit_x_load(*order[i])
            nxt = [3]
            qkv_idx = [1]

            emit_qkv_qk(*order[0])
            emit_qkv_v(*order[0])
            for b in range(B):
                for w in range(NW):
                    av, lrow = emit_window(b, w)
                    have_qkv = qkv_idx[0] < len(order)
                    if have_qkv:       # QKV Q/K mms: PE filler for the tail
                        emit_qkv_qk(*order[qkv_idx[0]])
                    st_n1, st_n2 = emit_bc(b, w, lrow)
                    if have_qkv:
                        emit_qkv_v(*order[qkv_idx[0]])
                        qkv_idx[0] += 1
                        if nxt[0] < len(order):
                            emit_x_load(*order[nxt[0]])
                            nxt[0] += 1
                    emit_norm_finish(b, w, av, st_n1, st_n2)
                    push_proj(b, w)
            while deferred:
                deferred.pop(0)()
    nc.compile()
    return nc


_PROGRAM = None


def _get_program():
    global _PROGRAM
    if _PROGRAM is None:
        _PROGRAM = build_program()
    return _PROGRAM


def _make_in_maps(x, W_qkv, b_qkv, W_proj):
    B, T, C = x.shape
    xT = np.ascontiguousarray(x.reshape(B * T, C).T.astype(np.float32))
    in_maps = []
    for c in range(N_CORES):
        lo, hi = c * D_LOC, (c + 1) * D_LOC
        wp_swapped = np.concatenate(
            [W_proj[lo + 64:hi, :], W_proj[lo:lo + 64, :]], axis=0
        )
        in_maps.append({
            "xT": xT,
            "wq": np.ascontiguousarray(W_qkv[:, lo:hi], np.float32),
            "wk": np.ascontiguousarray(W_qkv[:, C + lo:C + hi], np.float32),
            "wv": np.ascontiguousarray(W_qkv[:, 2 * C + lo:2 * C + hi], np.float32),
            "bq": np.ascontiguousarray(b_qkv[lo:hi].reshape(-1, 1), np.float32),
            "bk": np.ascontiguousarray(b_qkv[C + lo:C + hi].reshape(-1, 1), np.float32),
            "bv": np.ascontiguousarray(b_qkv[2 * C + lo:2 * C + hi].reshape(-1, 1), np.float32),
            "wp": np.ascontiguousarray(wp_swapped, np.float32),
        })
    return in_maps


LAST_RESULT = None


def run(inputs, trace=False):
    """Returns (full output [B,T,C] float32, exec_time_ns or None)."""
    global LAST_RESULT
    x = np.asarray(inputs["x"], np.float32)
    W_qkv = np.asarray(inputs["W_qkv"], np.float32)
    b_qkv = np.asarray(inputs["b_qkv"], np.float32)
    W_proj = np.asarray(inputs["W_proj"], np.float32)
    b_proj = np.asarray(inputs["b_proj"], np.float32)
    B, T, C = x.shape

    nc = _get_program()
    in_maps = _make_in_maps(x, W_qkv, b_qkv, W_proj)
    res = run_bass_kernel_spmd(nc, in_maps, list(range(N_CORES)), trace=trace)
    LAST_RESULT = res
    acc = np.zeros((B * T, C), np.float32)
    for c in range(N_CORES):
        acc += np.asarray(res.results[c]["outp"]).astype(np.float32)
    out = acc + b_proj.astype(np.float32)
    return out.reshape(B, T, C), res.exec_time_ns


def kernel(**inputs):
    out, _ = run(inputs, trace=False)
    return out


# revision 11
# speedup vs baseline: 1.3048x; 1.3048x over previous
"""Causal self-attention (B=4, T=2048, C=1024, H=16) on 8 trn2 NeuronCores.

Sharding: tensor-parallel over heads. Core c owns heads {2c, 2c+1}:
  - computes Q,K,V projections for its 2 heads (full batch/sequence),
  - causal attention for its heads,
  - a partial output projection (row-slice of W_proj),
and the host sums the 8 partial projections (+ b_proj).

This version is built around keeping the PE (tensor engine) stream stall-free
so it ramps to and stays at full clock (back-to-back 512-row fp32r matmuls
measure ~230ns on this part):

  - All matmuls are fp32r (1 cycle/row at N>=256; bf16 is no faster on this
    hardware, measured) except the output staging, which is written bf16 to
    halve the output DMA (accuracy measured at ~1.7e-3 l2 vs 2e-2 budget).
  - Attention is software-pipelined: S^T of k-tile i+1 is issued before AV of
    k-tile i, so the exp (Activation engine) of tile i overlaps the S matmuls
    of tile i+1 on the PE.
  - Both heads are packed in one [128, 2, 512] S^T/P tile -> one exp per
    k-tile covers both heads.
  - Causal masking of diagonal tiles is done by zeroing the upper triangle of
    P AFTER exp with gpsimd affine_select (gpsimd is otherwise idle), keeping
    mask work off the DVE and scalar engines.
  - The softmax denominator l comes from a shared ones-column in the
    token-major V tile (layout [ones, V_h0, V_h1, ones], 130 cols): head0's
    AV uses cols 0:65 -> l lands on psum partition 63, O on 64:127; head1's
    uses cols 65:130 -> O on 0:63, l on 64. No cross-partition moves needed.
  - 1/l via the fast approximate-reciprocal DVE op on the single l row, then
    a K=1 ones-matmul broadcast across partitions.
  - QKV projection for batch b+1 is emitted between the attention windows of
    batch b: it fills the PE while the window-end normalize chain (DVE) runs.
  - The per-window output projection is split into 4 deferred pieces that are
    drip-fed into the next window's k-loop (1 piece per k-tile iteration), so
    the PSUM->SBUF staging copies never gate the PE.
  - PSUM budget (8 banks): stt ring 2x[128,2,512] (4) + av_h0/av_h1 (2) +
    po ring 2x[128,512] (2). The normalize broadcast and the V transposes
    borrow stt-ring slots.
"""

import numpy as np

import concourse.bacc as bacc
import concourse.bass as bass
import concourse.tile as tile
from concourse import mybir
from concourse.bass_utils import run_bass_kernel_spmd
from concourse.masks import make_identity

F32 = mybir.dt.float32
F32R = mybir.dt.float32r
BF16 = mybir.dt.bfloat16
AF = mybir.ActivationFunctionType
ALU = mybir.AluOpType

N_CORES = 8
D_MODEL = 1024
HEAD_DIM = 64
H_LOC = 2                  # heads per core
D_LOC = H_LOC * HEAD_DIM   # 128
SCALE = 1.0 / np.sqrt(HEAD_DIM)


def build_program(B=4, T=2048):
    TOK = B * T
    CT = D_MODEL // 128    # contraction tiles for the QKV matmul
    NTT = T // 512         # 512-token tiles per batch
    NW = T // 512          # q-windows per batch (512 wide)
    NKB = T // 128         # k-tiles per batch
    assert T % 512 == 0

    nc = bacc.Bacc(
        "TRN2", target_bir_lowering=False, debug=False, num_devices=N_CORES
    )
    xT = nc.dram_tensor("xT", [D_MODEL, TOK], F32R, kind="ExternalInput").ap()
    wq = nc.dram_tensor("wq", [D_MODEL, D_LOC], F32R, kind="ExternalInput").ap()
    wk = nc.dram_tensor("wk", [D_MODEL, D_LOC], F32R, kind="ExternalInput").ap()
    wv = nc.dram_tensor("wv", [D_MODEL, D_LOC], F32R, kind="ExternalInput").ap()
    bq = nc.dram_tensor("bq", [D_LOC, 1], F32, kind="ExternalInput").ap()
    bk = nc.dram_tensor("bk", [D_LOC, 1], F32, kind="ExternalInput").ap()
    bv = nc.dram_tensor("bv", [D_LOC, 1], F32, kind="ExternalInput").ap()
    # wp rows are pre-swapped on host: [head1 dims, head0 dims]
    wp = nc.dram_tensor("wp", [D_LOC, D_MODEL], F32R, kind="ExternalInput").ap()
    outp = nc.dram_tensor("outp", [TOK, D_MODEL], BF16, kind="ExternalOutput").ap()

    with tile.TileContext(nc) as tc:
        with (
            tc.tile_pool(name="const", bufs=1) as const,
            tc.tile_pool(name="res", bufs=1) as res,
            tc.tile_pool(name="xp", bufs=3) as xp,
            tc.tile_pool(name="vtp", bufs=2) as vtp,
            tc.tile_pool(name="ptp", bufs=3) as ptp,
            tc.tile_pool(name="lnp", bufs=2) as lnp,
            tc.tile_pool(name="obp", bufs=4) as obp,
            tc.tile_pool(name="pst", bufs=1, space="PSUM") as pst,
            tc.tile_pool(name="pav", bufs=1, space="PSUM") as pav,
            tc.tile_pool(name="ppo", bufs=1, space="PSUM") as ppo,
        ):
            # --- constants -------------------------------------------------
            wq_sb = const.tile([128, CT, D_LOC], F32R, tag="wq")
            wk_sb = const.tile([128, CT, D_LOC], F32R, tag="wk")
            wv_sb = const.tile([128, CT, D_LOC], F32R, tag="wv")
            for w_sb, w_dram in ((wq_sb, wq), (wk_sb, wk), (wv_sb, wv)):
                nc.sync.dma_start(
                    out=w_sb, in_=w_dram.rearrange("(ct p) d -> p ct d", p=128)
                )
            wp_sb = const.tile([128, D_MODEL], F32R, tag="wp")
            nc.sync.dma_start(out=wp_sb, in_=wp)
            bq_sb = const.tile([128, 1], F32, tag="bq")
            bk_sb = const.tile([128, 1], F32, tag="bk")
            bv_sb = const.tile([128, 1], F32, tag="bv")
            for b_sb, b_dram in ((bq_sb, bq), (bk_sb, bk), (bv_sb, bv)):
                nc.sync.dma_start(out=b_sb, in_=b_dram)

            ident_f32 = const.tile([128, 128], F32, tag="ident_f32")
            make_identity(nc, ident_f32)
            ident = const.tile([128, 128], F32R, tag="ident")
            nc.vector.tensor_copy(ident, ident_f32)
            ones_f32 = const.tile([128, 128], F32, tag="ones_f32")
            nc.vector.memset(ones_f32, 1.0)
            ones_r = const.tile([128, 128], F32R, tag="ones_r")
            nc.vector.tensor_copy(ones_r, ones_f32)
            # 0/1 causal mask for diagonal S^T blocks: keep where q - k >= 0
            tri_f32 = const.tile([128, 128], F32, tag="tri_f32")
            nc.vector.memset(tri_f32, 1.0)
            nc.gpsimd.affine_select(
                out=tri_f32,
                in_=tri_f32,
                compare_op=ALU.is_ge,
                fill=0.0,
                base=0,
                pattern=[[1, 128]],
                channel_multiplier=-1,
            )
            trimask = const.tile([128, 128], F32R, tag="trimask")
            nc.vector.tensor_copy(trimask, tri_f32)

            # --- per-batch residents (double-buffered) ---------------------
            qt = [res.tile([128, T], F32R, tag=f"qt{i}", name=f"qt{i}") for i in range(2)]
            kt = [res.tile([128, T], F32R, tag=f"kt{i}", name=f"kt{i}") for i in range(2)]
            # token-major V: [tok, kblk, 130] = [V_h0 (64), ones, V_h1 (64), ones]
            vtm = [res.tile([128, NKB, 130], F32R, tag=f"vtm{i}", name=f"vtm{i}") for i in range(2)]
            ot = [res.tile([128, T], F32R, tag=f"ot{i}", name=f"ot{i}") for i in range(2)]
            for i in range(2):
                nc.vector.tensor_copy(vtm[i][:, :, 64], ones_f32[:, 0:NKB])
                nc.vector.tensor_copy(vtm[i][:, :, 129], ones_f32[:, 0:NKB])

            x_tiles = {}

            def emit_x_load(b, tt):
                g = b * NTT + tt
                xt = xp.tile([128, CT, 512], F32R, tag="x", name=f"x_{g}")
                t0 = b * T + tt * 512
                for ct in range(CT):
                    nc.sync.dma_start(
                        out=xt[:, ct, :],
                        in_=xT[ct * 128:(ct + 1) * 128, t0:t0 + 512],
                    )
                x_tiles[g] = xt

            # One unified 6-deep ring of 1-bank [128, 512] psum tiles serves
            # every transient matmul target: per-head S tiles, the l
            # broadcasts, QKV accumulators, V transposes, and the two
            # projection halves. Depth 6 keeps the PE far ahead of the
            # consumers that release the slots.
            def st_tile(name):
                return pst.tile([128, 512], F32, tag="st", bufs=6, name=name)

            def emit_qkv_qk(b, tt):
                bb = b % 2
                xt = x_tiles[b * NTT + tt]
                t0 = tt * 512
                tq = st_tile(f"tq_{b}_{tt}")
                for ct in range(CT):
                    nc.tensor.matmul(
                        tq, wq_sb[:, ct, :], xt[:, ct, :],
                        start=(ct == 0), stop=(ct == CT - 1),
                    )
                nc.vector.tensor_scalar_add(qt[bb][:, t0:t0 + 512], tq, bq_sb)
                tk = st_tile(f"tk_{b}_{tt}")
                for ct in range(CT):
                    nc.tensor.matmul(
                        tk, wk_sb[:, ct, :], xt[:, ct, :],
                        start=(ct == 0), stop=(ct == CT - 1),
                    )
                nc.vector.tensor_scalar_add(kt[bb][:, t0:t0 + 512], tk, bk_sb)

            def emit_qkv_v(b, tt):
                bb = b % 2
                xt = x_tiles.pop(b * NTT + tt)
                tv = st_tile(f"tv_{b}_{tt}")
                for ct in range(CT):
                    nc.tensor.matmul(
                        tv, wv_sb[:, ct, :], xt[:, ct, :],
                        start=(ct == 0), stop=(ct == CT - 1),
                    )
                vt = vtp.tile([128, 512], F32R, tag="vt", name=f"vt_{b}_{tt}")
                nc.vector.tensor_scalar_add(vt, tv, bv_sb)
                st = st_tile(f"tp_{b}_{tt}")
                for j in range(4):
                    nc.tensor.transpose(
                        st[:, j * 128:(j + 1) * 128].bitcast(F32R),
                        vt[:, j * 128:(j + 1) * 128],
                        ident,
                    )
                tpv = st.bitcast(F32R).rearrange("p (j q) -> p j q", j=4)
                nc.vector.tensor_copy(
                    vtm[bb][:, tt * 4:(tt + 1) * 4, 0:64], tpv[:, :, 0:64]
                )
                nc.vector.tensor_copy(
                    vtm[bb][:, tt * 4:(tt + 1) * 4, 65:129], tpv[:, :, 64:128]
                )

            deferred = []

            def push_proj(b, w):
                bb = b % 2
                for ti in range(4):
                    def piece(b=b, w=w, ti=ti, bb=bb):
                        ob = obp.tile([128, 1024], BF16, tag="ob",
                                      name=f"ob_{b}_{w}_{ti}")
                        t0 = w * 512 + ti * 128
                        for co in range(2):
                            t = st_tile(f"po_{b}_{w}_{ti}_{co}")
                            nc.tensor.matmul(
                                t,
                                ot[bb][:, t0:t0 + 128],
                                wp_sb[:, co * 512:(co + 1) * 512],
                                start=True, stop=True,
                            )
                            nc.vector.tensor_copy(
                                ob[:, co * 512:(co + 1) * 512], t
                            )
                        nc.sync.dma_start(
                            out=outp[b * T + t0:b * T + t0 + 128, :], in_=ob
                        )
                    deferred.append(piece)

            def emit_window(b, w):
                """S/exp/AV loop for q-window [w*512, (w+1)*512) of batch b.
                Per-head S tiles/exps; AV lags S by two k-tiles so exp (ACT)
                and the diagonal mask (gpsimd) latencies stay hidden."""
                bb = b % 2
                nk = 4 * (w + 1)
                q0 = w * 512
                av = pst.tile([128, 2, 512], F32, tag="av", bufs=1,
                              name=f"av_{b}_{w}")

                def emit_av(ki, off, pt):
                    stf, spf = (ki == 0), (ki == nk - 1)
                    nc.tensor.matmul(
                        av[0:65, 0, off:512],
                        vtm[bb][:, ki, 0:65],
                        pt[:, 0, off:512],
                        start=stf, stop=spf,
                    )
                    nc.tensor.matmul(
                        av[0:65, 1, off:512],
                        vtm[bb][:, ki, 65:130],
                        pt[:, 1, off:512],
                        start=stf, stop=spf,
                    )

                pending = []
                for ki in range(nk):
                    off = max(0, ki * 128 - q0)
                    diag = ki * 128 >= q0
                    pt = ptp.tile([128, 2, 512], F32R, tag="pt",
                                  name=f"pt_{b}_{w}_{ki}")
                    for h in range(2):
                        sth = st_tile(f"st_{b}_{w}_{ki}_{h}")
                        nc.tensor.matmul(
                            sth[:, off:512],
                            kt[bb][64 * h:64 * h + 64, ki * 128:(ki + 1) * 128],
                            qt[bb][64 * h:64 * h + 64, q0 + off:q0 + 512],
                            start=True, stop=True,
                        )
                        nc.scalar.activation(
                            pt[:, h, off:512], sth[:, off:512], AF.Exp,
                            scale=SCALE,
                        )
                        if diag:  # diagonal block: zero where q < k
                            nc.gpsimd.tensor_mul(
                                pt[:, h, off:off + 128],
                                pt[:, h, off:off + 128],
                                trimask,
                            )
                    pending.append((ki, off, pt))
                    if len(pending) > 2:
                        emit_av(*pending.pop(0))
                    if deferred:
                        deferred.pop(0)()
                while pending:
                    emit_av(*pending.pop(0))

                lrow = lnp.tile([128, 2, 512], F32R, tag="lr", name=f"lr_{b}_{w}")
                nc.vector.tensor_copy(lrow[64:65, 0, :], av[64:65, 0, :])
                nc.vector.tensor_copy(lrow[64:65, 1, :], av[64:65, 1, :])
                return av, lrow

            def emit_bc(b, w, lrow):
                # broadcast l to partitions 0:64 (slot n1: head1, n2: head0)
                st_n1 = st_tile(f"stn1_{b}_{w}")
                st_n2 = st_tile(f"stn2_{b}_{w}")
                nc.tensor.matmul(
                    st_n1[0:64, :], ones_r[64:65, 0:64], lrow[64:65, 1, :],
                    start=True, stop=True,
                )
                nc.tensor.matmul(
                    st_n2[0:64, :], ones_r[64:65, 0:64], lrow[64:65, 0, :],
                    start=True, stop=True,
                )
                return st_n1, st_n2

            def emit_norm_finish(b, w, av, st_n1, st_n2):
                bb = b % 2
                q0 = w * 512
                linv_sb = lnp.tile([128, 2, 512], F32, tag="ls", name=f"ls_{b}_{w}")
                nc.vector.reciprocal_approx_fast(
                    out=linv_sb[0:64, 0, :], in_=st_n1[0:64, :]
                )
                nc.vector.reciprocal_approx_fast(
                    out=linv_sb[0:64, 1, :], in_=st_n2[0:64, :]
                )
                nc.vector.tensor_mul(
                    ot[bb][0:64, q0:q0 + 512], av[0:64, 1, :], linv_sb[0:64, 0, :]
                )
                stg = lnp.tile([64, 512], F32R, tag="stg", name=f"stg_{b}_{w}")
                nc.vector.tensor_mul(
                    stg, av[0:64, 0, :], linv_sb[0:64, 1, :]
                )
                # cross-partition move: head0 O -> ot partitions 64:128,
                # split into 4 DMAs so the first proj piece isn't gated long
                for j in range(4):
                    nc.sync.dma_start(
                        out=ot[bb][64:128, q0 + j * 128:q0 + (j + 1) * 128],
                        in_=stg[:, j * 128:(j + 1) * 128],
                    )

            # ===================== schedule ================================
            # QKV unit g = 4*b + tt is consumed just before it is first
            # needed: unit 0 in the prologue, unit g at the end of attention
            # window g-1 (window (b, w) only reads tts 0..w of batch b).
            order = [(b, tt) for b in range(B) for tt in range(NTT)]
            for i in range(min(3, len(order))):
                emit_x_load(*order[i])
            nxt = [3]
            qkv_idx = [1]

            emit_qkv_qk(*order[0])
            emit_qkv_v(*order[0])
            for b in range(B):
                for w in range(NW):
                    av, lrow = emit_window(b, w)
                    have_qkv = qkv_idx[0] < len(order)
                    if have_qkv:       # QKV mms: PE filler for the tail
                        emit_qkv_qk(*order[qkv_idx[0]])
                    st_n1, st_n2 = emit_bc(b, w, lrow)
                    if have_qkv:
                        emit_qkv_v(*order[qkv_idx[0]])
                        qkv_idx[0] += 1
                        if nxt[0] < len(order):
                            emit_x_load(*order[nxt[0]])
                            nxt[0] += 1
                    emit_norm_finish(b, w, av, st_n1, st_n2)
                    push_proj(b, w)
            while deferred:
                deferred.pop(0)()
    nc.compile()
    return nc


_PROGRAM = None


def _get_program():
    global _PROGRAM
    if _PROGRAM is None:
        _PROGRAM = build_program()
    return _PROGRAM


def _make_in_maps(x, W_qkv, b_qkv, W_proj):
    B, T, C = x.shape
    xT = np.ascontiguousarray(x.reshape(B * T, C).T.astype(np.float32))
    in_maps = []
    for c in range(N_CORES):
        lo, hi = c * D_LOC, (c + 1) * D_LOC
        wp_swapped = np.concatenate(
            [W_proj[lo + 64:hi, :], W_proj[lo:lo + 64, :]], axis=0
        )
        in_maps.append({
            "xT": xT,
            "wq": np.ascontiguousarray(W_qkv[:, lo:hi], np.float32),
            "wk": np.ascontiguousarray(W_qkv[:, C + lo:C + hi], np.float32),
            "wv": np.ascontiguousarray(W_qkv[:, 2 * C + lo:2 * C + hi], np.float32),
            "bq": np.ascontiguousarray(b_qkv[lo:hi].reshape(-1, 1), np.float32),
            "bk": np.ascontiguousarray(b_qkv[C + lo:C + hi].reshape(-1, 1), np.float32),
            "bv": np.ascontiguousarray(b_qkv[2 * C + lo:2 * C + hi].reshape(-1, 1), np.float32),
            "wp": np.ascontiguousarray(wp_swapped, np.float32),
        })
    return in_maps


LAST_RESULT = None


def run(inputs, trace=False):
    """Returns (full output [B,T,C] float32, exec_time_ns or None)."""
    global LAST_RESULT
    x = np.asarray(inputs["x"], np.float32)
    W_qkv = np.asarray(inputs["W_qkv"], np.float32)
    b_qkv = np.asarray(inputs["b_qkv"], np.float32)
    W_proj = np.asarray(inputs["W_proj"], np.float32)
    b_proj = np.asarray(inputs["b_proj"], np.float32)
    B, T, C = x.shape

    nc = _get_program()
    in_maps = _make_in_maps(x, W_qkv, b_qkv, W_proj)
    res = run_bass_kernel_spmd(nc, in_maps, list(range(N_CORES)), trace=trace)
    LAST_RESULT = res
    acc = np.zeros((B * T, C), np.float32)
    for c in range(N_CORES):
        acc += np.asarray(res.results[c]["outp"]).astype(np.float32)
    out = acc + b_proj.astype(np.float32)
    return out.reshape(B, T, C), res.exec_time_ns


def kernel(**inputs):
    out, _ = run(inputs, trace=False)
    return out
